# revision 13
# baseline (speedup 1.0000x reference)
"""Trainium2 Bass kernel for a cached Mistral transformer layer.

Strategy (8-way tensor parallel, single SPMD launch):
  - Wq/Wk/Wv head-sharded: core c computes Q heads [4c,4c+4) and KV head c
    for ALL tokens (GQA group g=4 maps q-head h to kv-head h//4 == c).
  - RMSNorm + transpose done on each core's OWN 256 token rows, then
    AllGather of the d-major activations -> every core has x^T [4096, 2048].
  - Attention fully local per core (its heads, all tokens).
  - o^T AllToAll -> each core gets ALL heads for its OWN tokens, then a
    full-Wo matmul on the 256 own rows. Residual add in fp32.
  - MLP: Wg/Wu column-sharded (1792 cols/core) over all tokens, h AllToAll
    -> full-Wd matmul on own rows. Residual add in fp32.
  - All matmuls in bf16 with fp32 PSUM accumulation; norms/softmax fp32.
"""

import numpy as np
import ml_dtypes

import concourse.bacc as bacc
import concourse.bass as bass
import concourse.mybir as mybir
from concourse.tile import TileContext
from concourse.bass_utils import run_bass_kernel_spmd

F32 = mybir.dt.float32
BF16 = mybir.dt.bfloat16
AX = mybir.AxisListType.X
AF = mybir.ActivationFunctionType
OP = mybir.AluOpType

# Model dims (overridable before build for small-scale simulation)
B = 2
S = 1024
H = 32
HD = 128
KVH = 8
MLP = 14336
EPS = 1e-5
ROPE_BASE = 10000.0
NCORE = 8
NEG = -1.0e30

bf16 = ml_dtypes.bfloat16


def _dims():
    DM = H * HD                    # 4096 model dim
    T = B * S                      # all tokens
    T_OWN = T // NCORE             # own token rows
    HQC = H // NCORE               # q heads per core
    MLPC = MLP // NCORE            # mlp cols per core
    return DM, T, T_OWN, HQC, MLPC


def build_nc():
    DM, T, T_OWN, HQC, MLPC = _dims()
    KT = DM // 128                 # 32 contraction tiles over model dim
    MT_OWN = T_OWN // 128          # own-token partition tiles
    NCH = T // 512                 # 512-token chunks (2 ranks each)
    QT = S // 128                  # query tiles per batch
    NVT = T // 128                 # token tiles (v storage)
    KTM = MLP // 128               # 112 contraction tiles over mlp dim
    MTM = MLPC // 128              # 14 mlp col tiles per core
    SCALE = float(1.0 / np.sqrt(HD))
    RG = [list(range(NCORE))]

    nc = bacc.Bacc("TRN2", num_devices=NCORE)

    # ---- parameters ----
    stm = nc.declare_dram_parameter("stm", [T_OWN, DM], F32, isOutput=False)
    wq = nc.declare_dram_parameter("wq", [KT, 128, HQC * 128], BF16, isOutput=False)
    wk = nc.declare_dram_parameter("wk", [KT, 128, 128], BF16, isOutput=False)
    wv = nc.declare_dram_parameter("wv", [KT, 128, 128], BF16, isOutput=False)
    wo = nc.declare_dram_parameter("wo", [KT, 128, DM], BF16, isOutput=False)
    wg = nc.declare_dram_parameter("wg", [MTM, 128, KT * 128], BF16, isOutput=False)
    wu = nc.declare_dram_parameter("wu", [MTM, 128, KT * 128], BF16, isOutput=False)
    wd = nc.declare_dram_parameter("wd", [KTM, 128, DM], BF16, isOutput=False)
    lnw1 = nc.declare_dram_parameter("lnw1", [128, KT], F32, isOutput=False)
    lnw2 = nc.declare_dram_parameter("lnw2", [128, KT], F32, isOutput=False)
    cosT = nc.declare_dram_parameter("cosT", [128, S], F32, isOutput=False)
    sinS = nc.declare_dram_parameter("sinS", [128, S], F32, isOutput=False)
    ident = nc.declare_dram_parameter("ident", [128, 128], BF16, isOutput=False)
    trimask = nc.declare_dram_parameter("trimask", [128, 128], F32, isOutput=False)
    out = nc.declare_dram_parameter("out", [T_OWN, DM], F32, isOutput=True)

    # ---- internal DRAM ----
    x1t_own = nc.dram_tensor("x1t_own", [DM, T_OWN], BF16)
    x1t_all = nc.dram_tensor("x1t_all", [NCORE, DM, T_OWN], BF16, addr_space="Shared")
    ot_in = nc.dram_tensor("ot_in", [NCORE, HQC * 128, T_OWN], BF16)
    ot_out = nc.dram_tensor("ot_out", [NCORE, HQC * 128, T_OWN], BF16)
    x2t_own = nc.dram_tensor("x2t_own", [DM, T_OWN], BF16)
    x2t_all = nc.dram_tensor("x2t_all", [NCORE, DM, T_OWN], BF16, addr_space="Shared")
    h_in = nc.dram_tensor("h_in", [NCORE, MLPC, T_OWN], BF16)
    h_out = nc.dram_tensor("h_out", [NCORE, MLPC, T_OWN], BF16)
    stm2d = nc.dram_tensor("stm2d", [T_OWN, DM], F32)

    with TileContext(nc) as tc:
        # ======== constants ========
        cpool = tc.alloc_tile_pool(name="const", bufs=1)
        ident_sb = cpool.tile([128, 128], BF16, tag="ident")
        nc.sync.dma_start(out=ident_sb[:], in_=ident[:])
        tri_sb = cpool.tile([128, 128], F32, tag="tri")
        nc.sync.dma_start(out=tri_sb[:], in_=trimask[:])
        cos_sb = cpool.tile([128, S], F32, tag="cos")
        nc.sync.dma_start(out=cos_sb[:], in_=cosT[:])
        sin_sb = cpool.tile([128, S], F32, tag="sin")
        nc.sync.dma_start(out=sin_sb[:], in_=sinS[:])
        lnw1_sb = cpool.tile([128, KT], F32, tag="lnw1")
        nc.sync.dma_start(out=lnw1_sb[:], in_=lnw1[:])
        lnw2_sb = cpool.tile([128, KT], F32, tag="lnw2")
        nc.sync.dma_start(out=lnw2_sb[:], in_=lnw2[:])

        # stm rows stay resident in fp32 until the attention residual add.
        stm_pool = tc.alloc_tile_pool(name="stm_res", bufs=1)
        stm_sb = [stm_pool.tile([128, DM], F32, tag=f"stm{m}", name=f"stm{m}") for m in range(MT_OWN)]
        for m in range(MT_OWN):
            nc.sync.dma_start(out=stm_sb[m][:], in_=stm[m * 128:(m + 1) * 128, :])

        # ---- shared helper: rmsnorm 128-row tile -> scale factor [128,1] ----
        def ln_transpose_phase(src_tiles, lnw_sb, dst_dram, tag):
            # src_tiles: MT_OWN fp32 [128, DM] token-major tiles
            with tc.tile_pool(name=f"{tag}_w", bufs=2) as wkp, \
                 tc.tile_pool(name=f"{tag}_xt", bufs=1) as xtp, \
                 tc.tile_pool(name=f"{tag}_ps", bufs=4, space="PSUM") as psp:
                xt_sb = [xtp.tile([128, T_OWN], BF16, tag=f"{tag}xt{k}", name=f"{tag}xt{k}") for k in range(KT)]
                for m in range(MT_OWN):
                    sq = wkp.tile([128, DM], BF16, tag="sq")
                    ss = wkp.tile([128, 1], F32, tag="ss")
                    nc.scalar.activation(sq[:], src_tiles[m][:], AF.Square, accum_out=ss[:])
                    vv = wkp.tile([128, 1], F32, tag="vv")
                    nc.vector.tensor_scalar(vv[:], ss[:], 1.0 / DM, EPS, OP.mult, OP.add)
                    sv = wkp.tile([128, 1], F32, tag="sv")
                    nc.scalar.sqrt(sv[:], vv[:])
                    sf = wkp.tile([128, 1], F32, tag="sf")
                    nc.vector.reciprocal(sf[:], sv[:])
                    x1 = wkp.tile([128, DM], BF16, tag="x1")
                    nc.vector.tensor_scalar_mul(x1[:], src_tiles[m][:], sf[:])
                    for kt in range(KT):
                        ps = psp.tile([128, 128], BF16, tag="tps")
                        nc.tensor.transpose(ps[:], x1[:, kt * 128:(kt + 1) * 128], ident_sb[:])
                        nc.vector.tensor_scalar_mul(
                            xt_sb[kt][:, m * 128:(m + 1) * 128], ps[:],
                            lnw_sb[:, kt:kt + 1])
                for kt in range(KT):
                    nc.sync.dma_start(out=dst_dram[kt * 128:(kt + 1) * 128, :], in_=xt_sb[kt][:])

        # ======== phase A: ln1 + transpose + allgather ========
        ln_transpose_phase(stm_sb, lnw1_sb, x1t_own, "ln1")
        nc.gpsimd.collective_compute(
            "AllGather", OP.bypass, ins=[x1t_own[:]], outs=[x1t_all[:]],
            replica_groups=RG)

        # ======== phase B: QKV projections + RoPE ========
        qkv_w = tc.alloc_tile_pool(name="qkv_w", bufs=1)
        wq_sb = qkv_w.tile([128, KT, HQC * 128], BF16, tag="wq_sb")
        wk_sb = qkv_w.tile([128, KT, 128], BF16, tag="wk_sb")
        wv_sb = qkv_w.tile([128, KT, 128], BF16, tag="wv_sb")
        for kt in range(KT):
            nc.sync.dma_start(out=wq_sb[:, kt, :], in_=wq[kt])
            nc.sync.dma_start(out=wk_sb[:, kt, :], in_=wk[kt])
            nc.sync.dma_start(out=wv_sb[:, kt, :], in_=wv[kt])

        qkv_sb = tc.alloc_tile_pool(name="qkv_sb", bufs=1)
        q_sb = [qkv_sb.tile([128, T], BF16, tag=f"q{h}", name=f"q{h}") for h in range(HQC)]
        k_sb = qkv_sb.tile([128, T], BF16, tag="k_sb")
        v_sb = qkv_sb.tile([128, NVT, 128], BF16, tag="v_sb")

        with tc.tile_pool(name="qkv_x", bufs=3) as xp, \
             tc.tile_pool(name="qkv_rope", bufs=2) as rp, \
             tc.tile_pool(name="qkv_ps", bufs=1, space="PSUM") as qps_pool:
            for ch in range(NCH):
                qps = [qps_pool.tile([128, 512], F32, tag=f"qps{h}", name=f"qps{h}") for h in range(HQC)]
                kps = qps_pool.tile([128, 512], F32, tag="kps")
                vps = qps_pool.tile([128, 512], F32, tag="vps")
                rpc = 512 // T_OWN  # rank blocks per 512-token chunk
                for kt in range(KT):
                    xt = xp.tile([128, 512], BF16, tag="xt")
                    for half in range(rpc):
                        r = rpc * ch + half
                        nc.sync.dma_start(
                            out=xt[:, half * T_OWN:(half + 1) * T_OWN],
                            in_=x1t_all[r, kt * 128:(kt + 1) * 128, :])
                    st = kt == 0
                    sp = kt == KT - 1
                    for h in range(HQC):
                        nc.tensor.matmul(
                            qps[h][:], wq_sb[:, kt, h * 128:(h + 1) * 128], xt[:],
                            start=st, stop=sp)
                    nc.tensor.matmul(kps[:], wk_sb[:, kt, :], xt[:], start=st, stop=sp)
                    # one accumulation group per psum bank: start only zeroes
                    # (lazily) the whole 2KB region, so only the first matmul
                    # into the bank may set start, only the last sets stop.
                    for m2 in range(4):
                        nc.tensor.matmul(
                            vps[:, m2 * 128:(m2 + 1) * 128],
                            xt[:, m2 * 128:(m2 + 1) * 128], wv_sb[:, kt, :],
                            start=(st and m2 == 0), stop=(sp and m2 == 3))
                # V eviction (token-major tiles)
                for m2 in range(4):
                    nc.scalar.copy(v_sb[:, ch * 4 + m2, :], vps[:, m2 * 128:(m2 + 1) * 128])
                # RoPE on Q/K: pos slice within batch
                p0 = (ch * 512) % S
                cs = cos_sb[:, p0:p0 + 512]
                sn = sin_sb[:, p0:p0 + 512]
                for ps, dst in [(qps[h], q_sb[h]) for h in range(HQC)] + [(kps, k_sb)]:
                    rot = rp.tile([128, 512], F32, tag="rot")
                    nc.vector.tensor_copy(rot[0:64, :], ps[64:128, :])
                    nc.vector.tensor_copy(rot[64:128, :], ps[0:64, :])
                    tmp = rp.tile([128, 512], F32, tag="tmp")
                    nc.vector.tensor_mul(tmp[:], ps[:], cs)
                    nc.vector.tensor_mul(rot[:], rot[:], sn)
                    nc.vector.tensor_add(dst[:, ch * 512:(ch + 1) * 512], tmp[:], rot[:])

        # ======== phase C: attention (own heads, all tokens) ========
        ot_sb_pool = tc.alloc_tile_pool(name="ot_sb", bufs=1)
        oT_sb = [ot_sb_pool.tile([128, T], BF16, tag=f"ot{h}", name=f"ot{h}") for h in range(HQC)]

        with tc.tile_pool(name="att_ps", bufs=2, space="PSUM") as scp, \
             tc.tile_pool(name="att_pt_ps", bufs=2, space="PSUM") as ptp_pool, \
             tc.tile_pool(name="att_o_ps", bufs=2, space="PSUM") as op_pool, \
             tc.tile_pool(name="att_sb", bufs=3) as ap:
            for b in range(B):
                for h in range(HQC):
                    for qt in range(QT):
                        kx = (qt + 1) * 128
                        sc = scp.tile([128, min(S, 1024)], F32, tag="sc")
                        q_off = b * S + qt * 128
                        n0 = 0
                        while n0 < kx:
                            n1 = min(kx, n0 + 512)
                            nc.tensor.matmul(
                                sc[:, n0:n1], q_sb[h][:, q_off:q_off + 128],
                                k_sb[:, b * S + n0:b * S + n1],
                                start=True, stop=True)
                            n0 = n1
                        nc.vector.tensor_add(sc[:, kx - 128:kx], sc[:, kx - 128:kx], tri_sb[:])
                        nmax = ap.tile([128, 1], F32, tag="nmax")
                        nc.vector.reduce_max(nmax[:], sc[:, :kx], axis=AX, negate=True)
                        nbias = ap.tile([128, 1], F32, tag="nbias")
                        nc.vector.tensor_scalar_mul(nbias[:], nmax[:], SCALE)
                        p_sb = ap.tile([128, min(S, 1024)], BF16, tag="p")
                        ssum = ap.tile([128, 1], F32, tag="ssum")
                        nc.scalar.activation(
                            p_sb[:, :kx], sc[:, :kx], AF.Exp,
                            bias=nbias[:], scale=SCALE, accum_out=ssum[:])
                        rsum = ap.tile([128, 1], F32, tag="rsum")
                        nc.vector.reciprocal(rsum[:], ssum[:])
                        nc.vector.tensor_scalar_mul(p_sb[:, :kx], p_sb[:, :kx], rsum[:])
                        ops = op_pool.tile([128, 128], F32, tag="ops")
                        for kt in range(qt + 1):
                            ptp = ptp_pool.tile([128, 128], BF16, tag="ptp")
                            nc.tensor.transpose(
                                ptp[:], p_sb[:, kt * 128:(kt + 1) * 128], ident_sb[:])
                            pt_sb = ap.tile([128, 128], BF16, tag="pt")
                            nc.vector.tensor_copy(pt_sb[:], ptp[:])
                            nc.tensor.matmul(
                                ops[:], v_sb[:, b * (S // 128) + kt, :], pt_sb[:],
                                start=(kt == 0), stop=(kt == qt))
                        nc.scalar.copy(oT_sb[h][:, q_off:q_off + 128], ops[:])

        # o^T -> AllToAll blocks (block j = own-token slice of dest rank j)
        for h in range(HQC):
            for j in range(NCORE):
                nc.sync.dma_start(
                    out=ot_in[j, h * 128:(h + 1) * 128, :],
                    in_=oT_sb[h][:, j * T_OWN:(j + 1) * T_OWN])
        nc.gpsimd.collective_compute(
            "AllToAll", OP.bypass, ins=[ot_in[:]], outs=[ot_out[:]],
            replica_groups=RG)
        ot_sb_pool.release()
        qkv_sb.release()
        qkv_w.release()

        # ======== phase D: attn_out = o^T.T @ Wo (own tokens, full Wo) ========
        stm2_pool = tc.alloc_tile_pool(name="stm2", bufs=1)
        stm2_sb = [stm2_pool.tile([128, DM], F32, tag=f"stm2_{m}", name=f"stm2_{m}") for m in range(MT_OWN)]

        with tc.tile_pool(name="otc", bufs=1) as otc_pool, \
             tc.tile_pool(name="wo_st", bufs=3) as wop, \
             tc.tile_pool(name="d_ps", bufs=1, space="PSUM") as dps:
            otc = [otc_pool.tile([128, T_OWN], BF16, tag=f"otc{kt}", name=f"otc{kt}") for kt in range(KT)]
            for kt in range(KT):
                nc.sync.dma_start(
                    out=otc[kt][:],
                    in_=ot_out[kt // HQC, (kt % HQC) * 128:((kt % HQC) + 1) * 128, :])
            for ng in range(2):
                psd = [[dps.tile([128, 512], F32, tag=f"dp{m}_{n}", name=f"dp{m}_{n}") for n in range(4)]
                       for m in range(MT_OWN)]
                for kt in range(KT):
                    wot = wop.tile([128, DM // 2], BF16, tag="wot")
                    nc.sync.dma_start(out=wot[:], in_=wo[kt, :, ng * (DM // 2):(ng + 1) * (DM // 2)])
                    st = kt == 0
                    sp = kt == KT - 1
                    for m in range(MT_OWN):
                        for n in range(4):
                            nc.tensor.matmul(
                                psd[m][n][:], otc[kt][:, m * 128:(m + 1) * 128],
                                wot[:, n * 512:(n + 1) * 512], start=st, stop=sp)
                for m in range(MT_OWN):
                    for n in range(4):
                        col = ng * (DM // 2) + n * 512
                        nc.vector.tensor_add(
                            stm2_sb[m][:, col:col + 512], psd[m][n][:],
                            stm_sb[m][:, col:col + 512])
        for m in range(MT_OWN):
            nc.sync.dma_start(out=stm2d[m * 128:(m + 1) * 128, :], in_=stm2_sb[m][:])

        # ======== phase E: ln2 + transpose + allgather ========
        ln_transpose_phase(stm2_sb, lnw2_sb, x2t_own, "ln2")
        nc.gpsimd.collective_compute(
            "AllGather", OP.bypass, ins=[x2t_own[:]], outs=[x2t_all[:]],
            replica_groups=RG)
        stm2_pool.release()
        stm_pool.release()

        # ======== phase F: gate/up + silu + h AllToAll ========
        with tc.tile_pool(name="x2c", bufs=1) as x2cp, \
             tc.tile_pool(name="gu_w", bufs=2) as guw, \
             tc.tile_pool(name="gu_h", bufs=3) as ghp, \
             tc.tile_pool(name="gu_ps", bufs=2, space="PSUM") as gup:
            x2c = [x2cp.tile([128, T], BF16, tag=f"x2c{kt}", name=f"x2c{kt}") for kt in range(KT)]
            for kt in range(KT):
                for r in range(NCORE):
                    nc.sync.dma_start(
                        out=x2c[kt][:, r * T_OWN:(r + 1) * T_OWN],
                        in_=x2t_all[r, kt * 128:(kt + 1) * 128, :])
            for mt in range(MTM):
                wgt = guw.tile([128, KT * 128], BF16, tag="wgt")
                nc.sync.dma_start(out=wgt[:], in_=wg[mt])
                wut = guw.tile([128, KT * 128], BF16, tag="wut")
                nc.sync.dma_start(out=wut[:], in_=wu[mt])
                for ntc in range(T // 512):
                    gps = gup.tile([128, 512], F32, tag="gps")
                    ups = gup.tile([128, 512], F32, tag="ups")
                    for kt in range(KT):
                        st = kt == 0
                        sp = kt == KT - 1
                        nc.tensor.matmul(
                            gps[:], wgt[:, kt * 128:(kt + 1) * 128],
                            x2c[kt][:, ntc * 512:(ntc + 1) * 512], start=st, stop=sp)
                        nc.tensor.matmul(
                            ups[:], wut[:, kt * 128:(kt + 1) * 128],
                            x2c[kt][:, ntc * 512:(ntc + 1) * 512], start=st, stop=sp)
                    sg = ghp.tile([128, 512], BF16, tag="sg")
                    nc.scalar.activation(sg[:], gps[:], AF.Sigmoid)
                    gg = ghp.tile([128, 512], BF16, tag="gg")
                    nc.vector.scalar_tensor_tensor(
                        gg[:], gps[:], 1.0, sg[:], OP.mult, OP.mult)
                    ht = ghp.tile([128, 512], BF16, tag="ht")
                    nc.vector.tensor_mul(ht[:], gg[:], ups[:])
                    piece = min(T_OWN, 512)
                    for half in range(512 // piece):
                        tok0 = ntc * 512 + half * piece
                        j = tok0 // T_OWN
                        joff = tok0 % T_OWN
                        nc.sync.dma_start(
                            out=h_in[j, mt * 128:(mt + 1) * 128, joff:joff + piece],
                            in_=ht[:, half * piece:(half + 1) * piece])
        nc.gpsimd.collective_compute(
            "AllToAll", OP.bypass, ins=[h_in[:]], outs=[h_out[:]],
            replica_groups=RG)

        # ======== phase G: out = h^T.T @ Wd + stm2 (own tokens, full Wd) ========
        with tc.tile_pool(name="hc", bufs=1) as hcp, \
             tc.tile_pool(name="wd_st", bufs=4) as wdp, \
             tc.tile_pool(name="g_out", bufs=4) as gop, \
             tc.tile_pool(name="g_ps", bufs=1, space="PSUM") as gps_pool:
            hc = [hcp.tile([128, T_OWN], BF16, tag=f"hc{kt}", name=f"hc{kt}") for kt in range(KTM)]
            for kt in range(KTM):
                nc.sync.dma_start(
                    out=hc[kt][:],
                    in_=h_out[kt // MTM, (kt % MTM) * 128:((kt % MTM) + 1) * 128, :])
            for ng in range(2):
                psg = [[gps_pool.tile([128, 512], F32, tag=f"gp{m}_{n}", name=f"gp{m}_{n}") for n in range(4)]
                       for m in range(MT_OWN)]
                for kt in range(KTM):
                    wdt = wdp.tile([128, DM // 2], BF16, tag="wdt")
                    nc.sync.dma_start(out=wdt[:], in_=wd[kt, :, ng * (DM // 2):(ng + 1) * (DM // 2)])
                    st = kt == 0
                    sp = kt == KTM - 1
                    for m in range(MT_OWN):
                        for n in range(4):
                            nc.tensor.matmul(
                                psg[m][n][:], hc[kt][:, m * 128:(m + 1) * 128],
                                wdt[:, n * 512:(n + 1) * 512], start=st, stop=sp)
                for m in range(MT_OWN):
                    for n in range(4):
                        col = ng * (DM // 2) + n * 512
                        s2 = gop.tile([128, 512], F32, tag="s2")
                        nc.sync.dma_start(
                            out=s2[:], in_=stm2d[m * 128:(m + 1) * 128, col:col + 512])
                        oo = gop.tile([128, 512], F32, tag="oo")
                        nc.vector.tensor_add(oo[:], psg[m][n][:], s2[:])
                        nc.sync.dma_start(
                            out=out[m * 128:(m + 1) * 128, col:col + 512], in_=oo[:])
        cpool.release()

    nc.compile()
    return nc


# ---------------- host-side prep ----------------

def _rope_tables():
    inv_freq = 1.0 / (ROPE_BASE ** (np.arange(0, HD, 2, dtype=np.float64) / HD))
    t = np.arange(S, dtype=np.float64)
    freqs = t[:, None] * inv_freq[None, :]          # [S, HD/2]
    emb = np.concatenate([freqs, freqs], axis=-1)   # [S, HD]
    return np.cos(emb).astype(np.float32), np.sin(emb).astype(np.float32)


def prep_in_maps(stm, Wq, Wk, Wv, Wo, Wg, Wu, Wd, w_ln1, w_ln2):
    DM, T, T_OWN, HQC, MLPC = _dims()
    KT = DM // 128
    KTM = MLP // 128
    MTM = MLPC // 128

    stm_flat = np.ascontiguousarray(np.asarray(stm, np.float32).reshape(T, DM))
    cos, sin = _rope_tables()
    cosT = np.ascontiguousarray(cos.T)                     # [128, S]
    sinT = sin.T.copy()
    sinT[:HD // 2] *= -1.0                                 # sign for rotate-half
    sinS = np.ascontiguousarray(sinT)
    identity = np.eye(128, dtype=np.float32).astype(bf16)
    tri = np.zeros((128, 128), np.float32)
    tri[np.triu_indices(128, 1)] = NEG

    wo_t = np.ascontiguousarray(np.asarray(Wo, np.float32).astype(bf16).reshape(KT, 128, DM))
    wd_t = np.ascontiguousarray(np.asarray(Wd, np.float32).astype(bf16).reshape(KTM, 128, DM))
    lnw1 = np.ascontiguousarray(np.asarray(w_ln1, np.float32).reshape(KT, 128).T)
    lnw2 = np.ascontiguousarray(np.asarray(w_ln2, np.float32).reshape(KT, 128).T)

    Wq = np.asarray(Wq, np.float32).astype(bf16)
    Wk = np.asarray(Wk, np.float32).astype(bf16)
    Wv = np.asarray(Wv, np.float32).astype(bf16)
    Wg = np.asarray(Wg, np.float32).astype(bf16)
    Wu = np.asarray(Wu, np.float32).astype(bf16)

    in_maps = []
    for c in range(NCORE):
        qs = slice(c * HQC * 128, (c + 1) * HQC * 128)
        kvs = slice(c * 128, (c + 1) * 128)
        ms = slice(c * MLPC, (c + 1) * MLPC)
        wq_c = np.ascontiguousarray(Wq[:, qs].reshape(KT, 128, HQC * 128))
        wk_c = np.ascontiguousarray(Wk[:, kvs].reshape(KT, 128, 128))
        wv_c = np.ascontiguousarray(Wv[:, kvs].reshape(KT, 128, 128))
        # [DM, MLPC] -> [MTM, 128, KT*128]
        wg_c = np.ascontiguousarray(
            Wg[:, ms].reshape(KT, 128, MTM, 128).transpose(2, 1, 0, 3).reshape(
                MTM, 128, KT * 128))
        wu_c = np.ascontiguousarray(
            Wu[:, ms].reshape(KT, 128, MTM, 128).transpose(2, 1, 0, 3).reshape(
                MTM, 128, KT * 128))
        in_maps.append({
            "stm": np.ascontiguousarray(stm_flat[c * T_OWN:(c + 1) * T_OWN]),
            "wq": wq_c, "wk": wk_c, "wv": wv_c, "wo": wo_t,
            "wg": wg_c, "wu": wu_c, "wd": wd_t,
            "lnw1": lnw1, "lnw2": lnw2,
            "cosT": cosT, "sinS": sinS,
            "ident": identity, "trimask": tri,
        })
    return in_maps


_NC_CACHE = {}


def get_nc():
    key = (B, S, H, HD, KVH, MLP)
    if key not in _NC_CACHE:
        _NC_CACHE[key] = build_nc()
    return _NC_CACHE[key]


def kernel(**inputs):
    DM, T, T_OWN, HQC, MLPC = _dims()
    nc = get_nc()
    in_maps = prep_in_maps(**inputs)
    res = run_bass_kernel_spmd(nc, in_maps, list(range(NCORE)))
    outs = [res.results[c]["out"] for c in range(NCORE)]
    full = np.concatenate(outs, axis=0)              # [T, DM]
    return np.ascontiguousarray(full.reshape(B, S, H, HD).astype(np.float32))


# revision 14
# speedup vs baseline: 37.0625x; 37.0625x over previous
"""Trainium2 Bass kernel for a cached Mistral transformer layer.

Strategy (8-way tensor parallel, single SPMD launch):
  - Wq/Wk/Wv head-sharded: core c computes Q heads [4c,4c+4) and KV head c
    for ALL tokens (GQA group g=4 maps q-head h to kv-head h//4 == c).
  - RMSNorm + transpose done on each core's OWN 256 token rows, then
    AllGather of the d-major activations -> every core has x^T [4096, 2048].
  - Attention fully local per core (its heads, all tokens).
  - o^T AllToAll -> each core gets ALL heads for its OWN tokens, then a
    full-Wo matmul on the 256 own rows. Residual add in fp32.
  - MLP: Wg/Wu column-sharded (1792 cols/core) over all tokens, h AllToAll
    -> full-Wd matmul on own rows. Residual add in fp32.
  - All matmuls in bf16 with fp32 PSUM accumulation; norms/softmax fp32.
"""

import numpy as np
import ml_dtypes

import concourse.bacc as bacc
import concourse.bass as bass
import concourse.mybir as mybir
from concourse.tile import TileContext
from concourse.bass_utils import run_bass_kernel_spmd

F32 = mybir.dt.float32
BF16 = mybir.dt.bfloat16
AX = mybir.AxisListType.X
AF = mybir.ActivationFunctionType
OP = mybir.AluOpType

# Model dims (overridable before build for small-scale simulation)
B = 2
S = 1024
H = 32
HD = 128
KVH = 8
MLP = 14336
EPS = 1e-5
ROPE_BASE = 10000.0
NCORE = 8
NEG = -1.0e30

bf16 = ml_dtypes.bfloat16


def _dims():
    DM = H * HD                    # 4096 model dim
    T = B * S                      # all tokens
    T_OWN = T // NCORE             # own token rows
    HQC = H // NCORE               # q heads per core
    MLPC = MLP // NCORE            # mlp cols per core
    return DM, T, T_OWN, HQC, MLPC


def build_nc(skip=frozenset()):
    DM, T, T_OWN, HQC, MLPC = _dims()
    KT = DM // 128                 # 32 contraction tiles over model dim
    MT_OWN = T_OWN // 128          # own-token partition tiles
    NCH = T // 512                 # 512-token chunks (2 ranks each)
    QT = S // 128                  # query tiles per batch
    NVT = T // 128                 # token tiles (v storage)
    KTM = MLP // 128               # 112 contraction tiles over mlp dim
    MTM = MLPC // 128              # 14 mlp col tiles per core
    SCALE = float(1.0 / np.sqrt(HD))
    RG = [list(range(NCORE))]

    nc = bacc.Bacc("TRN2", num_devices=NCORE)

    # ---- parameters ----
    stm = nc.declare_dram_parameter("stm", [T_OWN, DM], F32, isOutput=False)
    wq = nc.declare_dram_parameter("wq", [KT, 128, HQC * 128], BF16, isOutput=False)
    wk = nc.declare_dram_parameter("wk", [KT, 128, 128], BF16, isOutput=False)
    wv = nc.declare_dram_parameter("wv", [KT, 128, 128], BF16, isOutput=False)
    wo = nc.declare_dram_parameter("wo", [KT, 128, DM], BF16, isOutput=False)
    wg = nc.declare_dram_parameter("wg", [MTM, 128, KT * 128], BF16, isOutput=False)
    wu = nc.declare_dram_parameter("wu", [MTM, 128, KT * 128], BF16, isOutput=False)
    wd = nc.declare_dram_parameter("wd", [KTM, 128, DM], BF16, isOutput=False)
    lnw1 = nc.declare_dram_parameter("lnw1", [128, KT], F32, isOutput=False)
    lnw2 = nc.declare_dram_parameter("lnw2", [128, KT], F32, isOutput=False)
    cosT = nc.declare_dram_parameter("cosT", [128, S], F32, isOutput=False)
    sinS = nc.declare_dram_parameter("sinS", [128, S], F32, isOutput=False)
    ident = nc.declare_dram_parameter("ident", [128, 128], BF16, isOutput=False)
    trimask = nc.declare_dram_parameter("trimask", [128, 128], F32, isOutput=False)
    out = nc.declare_dram_parameter("out", [T_OWN, DM], F32, isOutput=True)

    # ---- internal DRAM ----
    x1t_own = nc.dram_tensor("x1t_own", [DM, T_OWN], BF16)
    x1t_all = nc.dram_tensor("x1t_all", [NCORE, DM, T_OWN], BF16, addr_space="Shared")
    ot_in = nc.dram_tensor("ot_in", [NCORE, HQC * 128, T_OWN], BF16)
    ot_out = nc.dram_tensor("ot_out", [NCORE, HQC * 128, T_OWN], BF16)
    x2t_own = nc.dram_tensor("x2t_own", [DM, T_OWN], BF16)
    x2t_all = nc.dram_tensor("x2t_all", [NCORE, DM, T_OWN], BF16, addr_space="Shared")
    h_in = nc.dram_tensor("h_in", [NCORE, MLPC, T_OWN], BF16)
    h_out = nc.dram_tensor("h_out", [NCORE, MLPC, T_OWN], BF16)
    stm2d = nc.dram_tensor("stm2d", [T_OWN, DM], F32)

    with TileContext(nc) as tc:
        # ======== constants ========
        cpool = tc.alloc_tile_pool(name="const", bufs=1)
        ident_sb = cpool.tile([128, 128], BF16, tag="ident")
        nc.sync.dma_start(out=ident_sb[:], in_=ident[:])
        tri_sb = cpool.tile([128, 128], F32, tag="tri")
        nc.sync.dma_start(out=tri_sb[:], in_=trimask[:])
        cos_sb = cpool.tile([128, S], F32, tag="cos")
        nc.sync.dma_start(out=cos_sb[:], in_=cosT[:])
        sin_sb = cpool.tile([128, S], F32, tag="sin")
        nc.sync.dma_start(out=sin_sb[:], in_=sinS[:])
        lnw1_sb = cpool.tile([128, KT], F32, tag="lnw1")
        nc.sync.dma_start(out=lnw1_sb[:], in_=lnw1[:])
        lnw2_sb = cpool.tile([128, KT], F32, tag="lnw2")
        nc.sync.dma_start(out=lnw2_sb[:], in_=lnw2[:])

        # stm rows stay resident in fp32 until the attention residual add.
        stm_pool = tc.alloc_tile_pool(name="stm_res", bufs=1)
        stm_sb = [stm_pool.tile([128, DM], F32, tag=f"stm{m}", name=f"stm{m}") for m in range(MT_OWN)]
        for m in range(MT_OWN):
            nc.sync.dma_start(out=stm_sb[m][:], in_=stm[m * 128:(m + 1) * 128, :])

        # ---- shared helper: rmsnorm 128-row tile -> scale factor [128,1] ----
        def ln_transpose_phase(src_tiles, lnw_sb, dst_dram, tag):
            # src_tiles: MT_OWN fp32 [128, DM] token-major tiles
            with tc.tile_pool(name=f"{tag}_w", bufs=2) as wkp, \
                 tc.tile_pool(name=f"{tag}_xt", bufs=1) as xtp, \
                 tc.tile_pool(name=f"{tag}_ps", bufs=4, space="PSUM") as psp:
                xt_sb = [xtp.tile([128, T_OWN], BF16, tag=f"{tag}xt{k}", name=f"{tag}xt{k}") for k in range(KT)]
                for m in range(MT_OWN):
                    sq = wkp.tile([128, DM], BF16, tag="sq")
                    ss = wkp.tile([128, 1], F32, tag="ss")
                    nc.scalar.activation(sq[:], src_tiles[m][:], AF.Square, accum_out=ss[:])
                    vv = wkp.tile([128, 1], F32, tag="vv")
                    nc.vector.tensor_scalar(vv[:], ss[:], 1.0 / DM, EPS, OP.mult, OP.add)
                    sv = wkp.tile([128, 1], F32, tag="sv")
                    nc.scalar.sqrt(sv[:], vv[:])
                    sf = wkp.tile([128, 1], F32, tag="sf")
                    nc.vector.reciprocal(sf[:], sv[:])
                    x1 = wkp.tile([128, DM], BF16, tag="x1")
                    nc.vector.tensor_scalar_mul(x1[:], src_tiles[m][:], sf[:])
                    for kt in range(KT):
                        ps = psp.tile([128, 128], BF16, tag="tps")
                        nc.tensor.transpose(ps[:], x1[:, kt * 128:(kt + 1) * 128], ident_sb[:])
                        nc.vector.tensor_scalar_mul(
                            xt_sb[kt][:, m * 128:(m + 1) * 128], ps[:],
                            lnw_sb[:, kt:kt + 1])
                for kt in range(KT):
                    nc.sync.dma_start(out=dst_dram[kt * 128:(kt + 1) * 128, :], in_=xt_sb[kt][:])

        # ======== phase A: ln1 + transpose + allgather ========
        ln_transpose_phase(stm_sb, lnw1_sb, x1t_own, "ln1")
        if "coll" in skip:
            nc.sync.dma_start(out=x1t_all[0], in_=x1t_own[:])
        else:
            nc.gpsimd.collective_compute(
                "AllGather", OP.bypass, ins=[x1t_own[:]], outs=[x1t_all[:]],
                replica_groups=RG)

        # ======== phase B: QKV projections + RoPE ========
        qkv_w = tc.alloc_tile_pool(name="qkv_w", bufs=1)
        wq_sb = qkv_w.tile([128, KT, HQC * 128], BF16, tag="wq_sb")
        wk_sb = qkv_w.tile([128, KT, 128], BF16, tag="wk_sb")
        wv_sb = qkv_w.tile([128, KT, 128], BF16, tag="wv_sb")
        for kt in range(KT):
            nc.sync.dma_start(out=wq_sb[:, kt, :], in_=wq[kt])
            nc.sync.dma_start(out=wk_sb[:, kt, :], in_=wk[kt])
            nc.sync.dma_start(out=wv_sb[:, kt, :], in_=wv[kt])

        qkv_sb = tc.alloc_tile_pool(name="qkv_sb", bufs=1)
        q_sb = [qkv_sb.tile([128, T], BF16, tag=f"q{h}", name=f"q{h}") for h in range(HQC)]
        k_sb = qkv_sb.tile([128, T], BF16, tag="k_sb")
        v_sb = qkv_sb.tile([128, NVT, 128], BF16, tag="v_sb")

        with tc.tile_pool(name="qkv_x", bufs=3) as xp, \
             tc.tile_pool(name="qkv_rope", bufs=2) as rp, \
             tc.tile_pool(name="qkv_ps", bufs=1, space="PSUM") as qps_pool:
            for ch in range(NCH):
                qps = [qps_pool.tile([128, 512], F32, tag=f"qps{h}", name=f"qps{h}") for h in range(HQC)]
                kps = qps_pool.tile([128, 512], F32, tag="kps")
                vps = qps_pool.tile([128, 512], F32, tag="vps")
                rpc = 512 // T_OWN  # rank blocks per 512-token chunk
                for kt in range(KT):
                    xt = xp.tile([128, 512], BF16, tag="xt")
                    for half in range(rpc):
                        r = rpc * ch + half
                        nc.sync.dma_start(
                            out=xt[:, half * T_OWN:(half + 1) * T_OWN],
                            in_=x1t_all[r, kt * 128:(kt + 1) * 128, :])
                    st = kt == 0
                    sp = kt == KT - 1
                    for h in range(HQC):
                        nc.tensor.matmul(
                            qps[h][:], wq_sb[:, kt, h * 128:(h + 1) * 128], xt[:],
                            start=st, stop=sp)
                    nc.tensor.matmul(kps[:], wk_sb[:, kt, :], xt[:], start=st, stop=sp)
                    # one accumulation group per psum bank: start only zeroes
                    # (lazily) the whole 2KB region, so only the first matmul
                    # into the bank may set start, only the last sets stop.
                    for m2 in range(4):
                        nc.tensor.matmul(
                            vps[:, m2 * 128:(m2 + 1) * 128],
                            xt[:, m2 * 128:(m2 + 1) * 128], wv_sb[:, kt, :],
                            start=(st and m2 == 0), stop=(sp and m2 == 3))
                # V eviction (token-major tiles)
                for m2 in range(4):
                    nc.scalar.copy(v_sb[:, ch * 4 + m2, :], vps[:, m2 * 128:(m2 + 1) * 128])
                # RoPE on Q/K: pos slice within batch
                p0 = (ch * 512) % S
                cs = cos_sb[:, p0:p0 + 512]
                sn = sin_sb[:, p0:p0 + 512]
                for ps, dst in [(qps[h], q_sb[h]) for h in range(HQC)] + [(kps, k_sb)]:
                    rot = rp.tile([128, 512], F32, tag="rot")
                    nc.vector.tensor_copy(rot[0:64, :], ps[64:128, :])
                    nc.vector.tensor_copy(rot[64:128, :], ps[0:64, :])
                    tmp = rp.tile([128, 512], F32, tag="tmp")
                    nc.vector.tensor_mul(tmp[:], ps[:], cs)
                    nc.vector.tensor_mul(rot[:], rot[:], sn)
                    nc.vector.tensor_add(dst[:, ch * 512:(ch + 1) * 512], tmp[:], rot[:])

        # ======== phase C: attention (own heads, all tokens) ========
        ot_sb_pool = tc.alloc_tile_pool(name="ot_sb", bufs=1)
        oT_sb = [ot_sb_pool.tile([128, T], BF16, tag=f"ot{h}", name=f"ot{h}") for h in range(HQC)]

        with tc.tile_pool(name="att_ps", bufs=2, space="PSUM") as scp, \
             tc.tile_pool(name="att_pt_ps", bufs=2, space="PSUM") as ptp_pool, \
             tc.tile_pool(name="att_o_ps", bufs=2, space="PSUM") as op_pool, \
             tc.tile_pool(name="att_sb", bufs=3) as ap:
            if "attn" in skip:
                for h in range(HQC):
                    nc.vector.memset(oT_sb[h][:], 0.0)
            for b in range(B if "attn" not in skip else 0):
                for h in range(HQC):
                    for qt in range(QT):
                        kx = (qt + 1) * 128
                        sc = scp.tile([128, min(S, 1024)], F32, tag="sc")
                        q_off = b * S + qt * 128
                        n0 = 0
                        while n0 < kx:
                            n1 = min(kx, n0 + 512)
                            nc.tensor.matmul(
                                sc[:, n0:n1], q_sb[h][:, q_off:q_off + 128],
                                k_sb[:, b * S + n0:b * S + n1],
                                start=True, stop=True)
                            n0 = n1
                        nc.vector.tensor_add(sc[:, kx - 128:kx], sc[:, kx - 128:kx], tri_sb[:])
                        nmax = ap.tile([128, 1], F32, tag="nmax")
                        nc.vector.reduce_max(nmax[:], sc[:, :kx], axis=AX, negate=True)
                        nbias = ap.tile([128, 1], F32, tag="nbias")
                        nc.vector.tensor_scalar_mul(nbias[:], nmax[:], SCALE)
                        p_sb = ap.tile([128, min(S, 1024)], BF16, tag="p")
                        ssum = ap.tile([128, 1], F32, tag="ssum")
                        nc.scalar.activation(
                            p_sb[:, :kx], sc[:, :kx], AF.Exp,
                            bias=nbias[:], scale=SCALE, accum_out=ssum[:])
                        rsum = ap.tile([128, 1], F32, tag="rsum")
                        nc.vector.reciprocal(rsum[:], ssum[:])
                        nc.vector.tensor_scalar_mul(p_sb[:, :kx], p_sb[:, :kx], rsum[:])
                        ops = op_pool.tile([128, 128], F32, tag="ops")
                        for kt in range(qt + 1):
                            ptp = ptp_pool.tile([128, 128], BF16, tag="ptp")
                            nc.tensor.transpose(
                                ptp[:], p_sb[:, kt * 128:(kt + 1) * 128], ident_sb[:])
                            pt_sb = ap.tile([128, 128], BF16, tag="pt")
                            nc.vector.tensor_copy(pt_sb[:], ptp[:])
                            nc.tensor.matmul(
                                ops[:], v_sb[:, b * (S // 128) + kt, :], pt_sb[:],
                                start=(kt == 0), stop=(kt == qt))
                        nc.scalar.copy(oT_sb[h][:, q_off:q_off + 128], ops[:])

        # o^T -> AllToAll blocks (block j = own-token slice of dest rank j)
        for h in range(HQC):
            for j in range(NCORE):
                nc.sync.dma_start(
                    out=ot_in[j, h * 128:(h + 1) * 128, :],
                    in_=oT_sb[h][:, j * T_OWN:(j + 1) * T_OWN])
        if "coll" in skip:
            nc.sync.dma_start(out=ot_out[0], in_=ot_in[0])
        else:
            nc.gpsimd.collective_compute(
                "AllToAll", OP.bypass, ins=[ot_in[:]], outs=[ot_out[:]],
                replica_groups=RG)
        ot_sb_pool.release()
        qkv_sb.release()
        qkv_w.release()

        # ======== phase D: attn_out = o^T.T @ Wo (own tokens, full Wo) ========
        stm2_pool = tc.alloc_tile_pool(name="stm2", bufs=1)
        stm2_sb = [stm2_pool.tile([128, DM], F32, tag=f"stm2_{m}", name=f"stm2_{m}") for m in range(MT_OWN)]

        with tc.tile_pool(name="otc", bufs=1) as otc_pool, \
             tc.tile_pool(name="wo_st", bufs=3) as wop, \
             tc.tile_pool(name="d_ps", bufs=1, space="PSUM") as dps:
            otc = [otc_pool.tile([128, T_OWN], BF16, tag=f"otc{kt}", name=f"otc{kt}") for kt in range(KT)]
            for kt in range(KT):
                nc.sync.dma_start(
                    out=otc[kt][:],
                    in_=ot_out[kt // HQC, (kt % HQC) * 128:((kt % HQC) + 1) * 128, :])
            for ng in range(2):
                psd = [[dps.tile([128, 512], F32, tag=f"dp{m}_{n}", name=f"dp{m}_{n}") for n in range(4)]
                       for m in range(MT_OWN)]
                for kt in range(KT):
                    wot = wop.tile([128, DM // 2], BF16, tag="wot")
                    nc.sync.dma_start(out=wot[:], in_=wo[kt, :, ng * (DM // 2):(ng + 1) * (DM // 2)])
                    st = kt == 0
                    sp = kt == KT - 1
                    for m in range(MT_OWN):
                        for n in range(4):
                            nc.tensor.matmul(
                                psd[m][n][:], otc[kt][:, m * 128:(m + 1) * 128],
                                wot[:, n * 512:(n + 1) * 512], start=st, stop=sp)
                for m in range(MT_OWN):
                    for n in range(4):
                        col = ng * (DM // 2) + n * 512
                        nc.vector.tensor_add(
                            stm2_sb[m][:, col:col + 512], psd[m][n][:],
                            stm_sb[m][:, col:col + 512])
        for m in range(MT_OWN):
            nc.sync.dma_start(out=stm2d[m * 128:(m + 1) * 128, :], in_=stm2_sb[m][:])

        # ======== phase E: ln2 + transpose + allgather ========
        ln_transpose_phase(stm2_sb, lnw2_sb, x2t_own, "ln2")
        if "coll" in skip:
            nc.sync.dma_start(out=x2t_all[0], in_=x2t_own[:])
        else:
            nc.gpsimd.collective_compute(
                "AllGather", OP.bypass, ins=[x2t_own[:]], outs=[x2t_all[:]],
                replica_groups=RG)
        stm2_pool.release()
        stm_pool.release()

        # ======== phase F: gate/up + silu + h AllToAll ========
        with tc.tile_pool(name="x2c", bufs=1) as x2cp, \
             tc.tile_pool(name="gu_w", bufs=2) as guw, \
             tc.tile_pool(name="gu_h", bufs=3) as ghp, \
             tc.tile_pool(name="gu_ps", bufs=2, space="PSUM") as gup:
            x2c = [x2cp.tile([128, T], BF16, tag=f"x2c{kt}", name=f"x2c{kt}") for kt in range(KT)]
            for kt in range(KT):
                for r in range(NCORE):
                    nc.sync.dma_start(
                        out=x2c[kt][:, r * T_OWN:(r + 1) * T_OWN],
                        in_=x2t_all[r, kt * 128:(kt + 1) * 128, :])
            for mt in range(MTM):
                wgt = guw.tile([128, KT * 128], BF16, tag="wgt")
                nc.sync.dma_start(out=wgt[:], in_=wg[mt])
                wut = guw.tile([128, KT * 128], BF16, tag="wut")
                nc.sync.dma_start(out=wut[:], in_=wu[mt])
                for ntc in range(T // 512):
                    if "gu" in skip:
                        htz = ghp.tile([128, 512], BF16, tag="ht")
                        nc.vector.memset(htz[:], 0.0)
                        piece = min(T_OWN, 512)
                        for half in range(512 // piece):
                            tok0 = ntc * 512 + half * piece
                            nc.sync.dma_start(
                                out=h_in[tok0 // T_OWN, mt * 128:(mt + 1) * 128,
                                         tok0 % T_OWN:tok0 % T_OWN + piece],
                                in_=htz[:, half * piece:(half + 1) * piece])
                        continue
                    gps = gup.tile([128, 512], F32, tag="gps")
                    ups = gup.tile([128, 512], F32, tag="ups")
                    for kt in range(KT):
                        st = kt == 0
                        sp = kt == KT - 1
                        nc.tensor.matmul(
                            gps[:], wgt[:, kt * 128:(kt + 1) * 128],
                            x2c[kt][:, ntc * 512:(ntc + 1) * 512], start=st, stop=sp)
                        nc.tensor.matmul(
                            ups[:], wut[:, kt * 128:(kt + 1) * 128],
                            x2c[kt][:, ntc * 512:(ntc + 1) * 512], start=st, stop=sp)
                    sg = ghp.tile([128, 512], BF16, tag="sg")
                    nc.scalar.activation(sg[:], gps[:], AF.Sigmoid)
                    gg = ghp.tile([128, 512], BF16, tag="gg")
                    nc.vector.scalar_tensor_tensor(
                        gg[:], gps[:], 1.0, sg[:], OP.mult, OP.mult)
                    ht = ghp.tile([128, 512], BF16, tag="ht")
                    nc.vector.tensor_mul(ht[:], gg[:], ups[:])
                    piece = min(T_OWN, 512)
                    for half in range(512 // piece):
                        tok0 = ntc * 512 + half * piece
                        j = tok0 // T_OWN
                        joff = tok0 % T_OWN
                        nc.sync.dma_start(
                            out=h_in[j, mt * 128:(mt + 1) * 128, joff:joff + piece],
                            in_=ht[:, half * piece:(half + 1) * piece])
        if "coll" in skip:
            nc.sync.dma_start(out=h_out[0], in_=h_in[0])
        else:
            nc.gpsimd.collective_compute(
                "AllToAll", OP.bypass, ins=[h_in[:]], outs=[h_out[:]],
                replica_groups=RG)

        # ======== phase G: out = h^T.T @ Wd + stm2 (own tokens, full Wd) ========
        with tc.tile_pool(name="hc", bufs=1) as hcp, \
             tc.tile_pool(name="wd_st", bufs=4) as wdp, \
             tc.tile_pool(name="g_out", bufs=4) as gop, \
             tc.tile_pool(name="g_ps", bufs=1, space="PSUM") as gps_pool:
            hc = [hcp.tile([128, T_OWN], BF16, tag=f"hc{kt}", name=f"hc{kt}") for kt in range(KTM)]
            for kt in range(KTM):
                nc.sync.dma_start(
                    out=hc[kt][:],
                    in_=h_out[kt // MTM, (kt % MTM) * 128:((kt % MTM) + 1) * 128, :])
            for ng in range(2):
                psg = [[gps_pool.tile([128, 512], F32, tag=f"gp{m}_{n}", name=f"gp{m}_{n}") for n in range(4)]
                       for m in range(MT_OWN)]
                for kt in range(KTM if "wd" not in skip else 0):
                    wdt = wdp.tile([128, DM // 2], BF16, tag="wdt")
                    nc.sync.dma_start(out=wdt[:], in_=wd[kt, :, ng * (DM // 2):(ng + 1) * (DM // 2)])
                    st = kt == 0
                    sp = kt == KTM - 1
                    for m in range(MT_OWN):
                        for n in range(4):
                            nc.tensor.matmul(
                                psg[m][n][:], hc[kt][:, m * 128:(m + 1) * 128],
                                wdt[:, n * 512:(n + 1) * 512], start=st, stop=sp)
                for m in range(MT_OWN):
                    for n in range(4):
                        col = ng * (DM // 2) + n * 512
                        s2 = gop.tile([128, 512], F32, tag="s2")
                        nc.sync.dma_start(
                            out=s2[:], in_=stm2d[m * 128:(m + 1) * 128, col:col + 512])
                        oo = gop.tile([128, 512], F32, tag="oo")
                        if "wd" in skip:
                            nc.vector.tensor_copy(oo[:], s2[:])
                        else:
                            nc.vector.tensor_add(oo[:], psg[m][n][:], s2[:])
                        nc.sync.dma_start(
                            out=out[m * 128:(m + 1) * 128, col:col + 512], in_=oo[:])
        cpool.release()

    nc.compile()
    return nc


# ---------------- host-side prep ----------------

def _rope_tables():
    inv_freq = 1.0 / (ROPE_BASE ** (np.arange(0, HD, 2, dtype=np.float64) / HD))
    t = np.arange(S, dtype=np.float64)
    freqs = t[:, None] * inv_freq[None, :]          # [S, HD/2]
    emb = np.concatenate([freqs, freqs], axis=-1)   # [S, HD]
    return np.cos(emb).astype(np.float32), np.sin(emb).astype(np.float32)


def prep_in_maps(stm, Wq, Wk, Wv, Wo, Wg, Wu, Wd, w_ln1, w_ln2):
    DM, T, T_OWN, HQC, MLPC = _dims()
    KT = DM // 128
    KTM = MLP // 128
    MTM = MLPC // 128

    stm_flat = np.ascontiguousarray(np.asarray(stm, np.float32).reshape(T, DM))
    cos, sin = _rope_tables()
    cosT = np.ascontiguousarray(cos.T)                     # [128, S]
    sinT = sin.T.copy()
    sinT[:HD // 2] *= -1.0                                 # sign for rotate-half
    sinS = np.ascontiguousarray(sinT)
    identity = np.eye(128, dtype=np.float32).astype(bf16)
    tri = np.zeros((128, 128), np.float32)
    tri[np.triu_indices(128, 1)] = NEG

    wo_t = np.ascontiguousarray(np.asarray(Wo, np.float32).astype(bf16).reshape(KT, 128, DM))
    wd_t = np.ascontiguousarray(np.asarray(Wd, np.float32).astype(bf16).reshape(KTM, 128, DM))
    lnw1 = np.ascontiguousarray(np.asarray(w_ln1, np.float32).reshape(KT, 128).T)
    lnw2 = np.ascontiguousarray(np.asarray(w_ln2, np.float32).reshape(KT, 128).T)

    Wq = np.asarray(Wq, np.float32).astype(bf16)
    Wk = np.asarray(Wk, np.float32).astype(bf16)
    Wv = np.asarray(Wv, np.float32).astype(bf16)
    Wg = np.asarray(Wg, np.float32).astype(bf16)
    Wu = np.asarray(Wu, np.float32).astype(bf16)

    in_maps = []
    for c in range(NCORE):
        qs = slice(c * HQC * 128, (c + 1) * HQC * 128)
        kvs = slice(c * 128, (c + 1) * 128)
        ms = slice(c * MLPC, (c + 1) * MLPC)
        wq_c = np.ascontiguousarray(Wq[:, qs].reshape(KT, 128, HQC * 128))
        wk_c = np.ascontiguousarray(Wk[:, kvs].reshape(KT, 128, 128))
        wv_c = np.ascontiguousarray(Wv[:, kvs].reshape(KT, 128, 128))
        # [DM, MLPC] -> [MTM, 128, KT*128]
        wg_c = np.ascontiguousarray(
            Wg[:, ms].reshape(KT, 128, MTM, 128).transpose(2, 1, 0, 3).reshape(
                MTM, 128, KT * 128))
        wu_c = np.ascontiguousarray(
            Wu[:, ms].reshape(KT, 128, MTM, 128).transpose(2, 1, 0, 3).reshape(
                MTM, 128, KT * 128))
        in_maps.append({
            "stm": np.ascontiguousarray(stm_flat[c * T_OWN:(c + 1) * T_OWN]),
            "wq": wq_c, "wk": wk_c, "wv": wv_c, "wo": wo_t,
            "wg": wg_c, "wu": wu_c, "wd": wd_t,
            "lnw1": lnw1, "lnw2": lnw2,
            "cosT": cosT, "sinS": sinS,
            "ident": identity, "trimask": tri,
        })
    return in_maps


_NC_CACHE = {}


def get_nc():
    key = (B, S, H, HD, KVH, MLP)
    if key not in _NC_CACHE:
        _NC_CACHE[key] = build_nc()
    return _NC_CACHE[key]


def kernel(**inputs):
    DM, T, T_OWN, HQC, MLPC = _dims()
    nc = get_nc()
    in_maps = prep_in_maps(**inputs)
    res = run_bass_kernel_spmd(nc, in_maps, list(range(NCORE)))
    outs = [res.results[c]["out"] for c in range(NCORE)]
    full = np.concatenate(outs, axis=0)              # [T, DM]
    return np.ascontiguousarray(full.reshape(B, S, H, HD).astype(np.float32))
